# revision 14
# baseline (speedup 1.0000x reference)
# Trainium2 Bass kernel for nn_CustomStyleLoss (segment-mean + MSE reduction).
#
# loss = sum_rows mean_chunks( (mean_chunk(input) - mean_chunk(style))^2 )
# with rows = 16*512 = 8192, each row = 50*50 = 2500 elems = 25 chunks of 100.
#
# Data-parallel over the row axis: core i gets rows [i*1024, (i+1)*1024).
# Raw Bass (no Tile framework). Per core the whole 20.5 MB fp32 shard fits
# in SBUF, so all DMAs are issued up-front with no buffer recycling: input
# pieces stream on the SP HWDGE ring, style pieces on the ACT ring; the 16
# SDMA engines drain both rings at the ~384 GB/s HBM-per-core limit
# (~53.4us for the full shard). Big 1.28MB tile DMAs are load-bearing:
# splitting the stream into half-tile DMAs measured ~20% slower (more,
# shorter descriptors), so only the LAST tile is split (2000+500 cols) to
# shorten the post-stream tail.
#
# Compute per piece: the DVE runs the fused subtract+prefix-scan
# (tensor_tensor_scan, the fastest single-pass fp32 primitive at ~2.15
# ns/elem), one drain (the scan does not flush before a dependent strided
# read), and one strided difference for the chunk sums. The
# square+accumulate runs on the otherwise-idle ACT engine (activation
# Square with accum_out into a per-piece partials column), removing two
# DVE ops per piece; cs has one slot per piece so there is no DVE/ACT
# buffer hazard. After the last 500-col piece lands (~62us) only ~1.5us
# of DVE work remains instead of a full 6.3us tile.

import sys

if "/opt/trn_rl_repo" not in sys.path:
    sys.path.insert(0, "/opt/trn_rl_repo")

import numpy as np

import concourse.bass as bass
from concourse import mybir
from concourse.bass_utils import run_bass_kernel_spmd

N_CORES = 8
N_ROWS = 8192          # 16 * 512
K = 2500               # 50 * 50
CHUNK = 100
P = 128
CPL = K // CHUNK                    # 25 chunks per row
ROWS_PER_CORE = N_ROWS // N_CORES   # 1024
N_TILES = ROWS_PER_CORE // P        # 8 tiles of [128 x 2500]
SPLIT = 2000                        # last tile: [0:2000] + [2000:2500]
PIECES = [(t, 0, K) for t in range(N_TILES - 1)] + [
    (N_TILES - 1, 0, SPLIT),
    (N_TILES - 1, SPLIT, K),
]
N_PIECES = len(PIECES)              # 9
W_MAX = K
SCALE = 1.0 / (CHUNK * np.sqrt(CPL))
SCALE2 = float(SCALE * SCALE)

_CACHED_NC = None


def _build_nc():
    nc = bass.Bass(
        "TRN2",
        target_bir_lowering=False,
        debug=False,
        num_devices=N_CORES,
    )
    x = nc.dram_tensor(
        "input", [ROWS_PER_CORE, K], mybir.dt.float32, kind="ExternalInput"
    ).ap()
    s = nc.dram_tensor(
        "style", [ROWS_PER_CORE, K], mybir.dt.float32, kind="ExternalInput"
    ).ap()
    o = nc.dram_tensor(
        "out", [P, N_PIECES], mybir.dt.float32, kind="ExternalOutput"
    ).ap()

    from contextlib import ExitStack

    with ExitStack() as ctx:
        xt = ctx.enter_context(
            nc.sbuf_tensor("xt", [P, N_TILES, K], mybir.dt.float32)
        )
        st = ctx.enter_context(
            nc.sbuf_tensor("st", [P, N_TILES, K], mybir.dt.float32)
        )
        # sc col 0 is a permanent zero so chunk sums are one strided sub.
        sc = ctx.enter_context(
            nc.sbuf_tensor("sc", [P, W_MAX + 1], mybir.dt.float32)
        )
        # cs one slot per piece: no reuse hazard between DVE and ACT.
        cs = ctx.enter_context(
            nc.sbuf_tensor("cs", [P, N_PIECES, CPL], mybir.dt.float32)
        )
        sq = ctx.enter_context(nc.sbuf_tensor("sq", [P, CPL], mybir.dt.float32))
        partials = ctx.enter_context(
            nc.sbuf_tensor("partials", [P, N_PIECES], mybir.dt.float32)
        )
        # One semaphore per DMA so no completion-ordering assumptions are
        # needed between DMAs on the same ring.
        s_in = [
            ctx.enter_context(nc.semaphore(f"s_in{i}")) for i in range(N_PIECES)
        ]
        s_st = [
            ctx.enter_context(nc.semaphore(f"s_st{i}")) for i in range(N_PIECES)
        ]
        s_d = ctx.enter_context(nc.semaphore("s_d"))
        s_cs = ctx.enter_context(nc.semaphore("s_cs"))
        s_out = ctx.enter_context(nc.semaphore("s_out"))
        block = ctx.enter_context(nc.Block(no_gpsimd_drain=True))

        def src(t_ap, piece):
            t, c0, c1 = piece
            return t_ap[t * P : (t + 1) * P, c0:c1]

        def dst(t_sb, piece):
            t, c0, c1 = piece
            return t_sb[:, t, c0:c1]

        @block.sync
        def _(sync):
            # Input pieces on the SP HWDGE ring; everything fits in SBUF so
            # all DMAs are queued immediately and drain back-to-back.
            for i, piece in enumerate(PIECES):
                sync.dma_start(out=dst(xt, piece), in_=src(x, piece)).then_inc(
                    s_in[i], 16
                )
            # Ship the per-core partial sums once all pieces are squared.
            sync.wait_ge(s_cs, N_PIECES)
            # No wait on the out-DMA receipt: the 4.6KB write lands in DRAM
            # within ~1us; the engine postamble + NRT teardown give it ample
            # time before the host reads the output.
            sync.dma_start(out=o, in_=partials[:]).then_inc(s_out, 16)

        @block.scalar
        def _(scalar):
            # Style pieces on the ACT HWDGE ring.
            for i, piece in enumerate(PIECES):
                scalar.dma_start(out=dst(st, piece), in_=src(s, piece)).then_inc(
                    s_st[i], 16
                )
            # Then the square+accumulate chain: partials[:, i] = sum_c cs^2
            # (SCALE^2 applied on the host).
            for i, piece in enumerate(PIECES):
                t, c0, c1 = piece
                nch = (c1 - c0) // CHUNK
                scalar.wait_ge(s_d, i + 1)
                nc.scalar.activation(
                    out=sq[:, 0:nch],
                    in_=cs[:, i, 0:nch],
                    func=mybir.ActivationFunctionType.Square,
                    accum_out=partials[:, i : i + 1],
                ).then_inc(s_cs, 1)

        @block.vector
        def _(vector):
            nc.vector.memset(sc[:, 0:1], 0.0)
            for i, piece in enumerate(PIECES):
                t, c0, c1 = piece
                w = c1 - c0
                nch = w // CHUNK
                vector.wait_ge(s_in[i], 16)
                vector.wait_ge(s_st[i], 16)
                # sc[:, j] = sum_{i<=j} (x - s) over this piece (fp32 state)
                nc.vector.tensor_tensor_scan(
                    out=sc[:, 1 : w + 1],
                    data0=dst(xt, piece),
                    data1=dst(st, piece),
                    initial=0.0,
                    op0=mybir.AluOpType.add,
                    op1=mybir.AluOpType.subtract,
                )
                # The scan does not flush before a dependent strided read.
                vector.drain()
                # chunk sums: cs[c] = S[100(c+1)] - S[100c]  (S[0] == 0)
                nc.vector.tensor_sub(
                    cs[:, i, 0:nch],
                    sc[:, CHUNK : w + 1 : CHUNK],
                    sc[:, 0:w:CHUNK],
                ).then_inc(s_d, 1)

    return nc


def _get_nc():
    global _CACHED_NC
    if _CACHED_NC is None:
        _CACHED_NC = _build_nc()
    return _CACHED_NC


def run_sharded(input, style, **run_kwargs):
    """Shard, run on 8 cores, return (scalar loss, BassKernelResults)."""
    nc = _get_nc()
    xi = np.ascontiguousarray(np.asarray(input, dtype=np.float32)).reshape(
        N_ROWS, K
    )
    xs = np.ascontiguousarray(np.asarray(style, dtype=np.float32)).reshape(
        N_ROWS, K
    )
    in_maps = [
        {
            "input": xi[i * ROWS_PER_CORE : (i + 1) * ROWS_PER_CORE],
            "style": xs[i * ROWS_PER_CORE : (i + 1) * ROWS_PER_CORE],
        }
        for i in range(N_CORES)
    ]
    res = run_bass_kernel_spmd(nc, in_maps, list(range(N_CORES)), **run_kwargs)
    total = np.float64(0.0)
    for r in res.results:
        total += r["out"].astype(np.float64).sum()
    return np.array(total * SCALE2, dtype=np.float32), res


def kernel(input, style):
    loss, _ = run_sharded(input, style)
    return loss


# revision 15
# speedup vs baseline: 1.0072x; 1.0072x over previous
# Trainium2 Bass kernel for nn_CustomStyleLoss (segment-mean + MSE reduction).
#
# loss = sum_rows mean_chunks( (mean_chunk(input) - mean_chunk(style))^2 )
# with rows = 16*512 = 8192, each row = 50*50 = 2500 elems = 25 chunks of 100.
#
# Data-parallel over the row axis: core i gets rows [i*1024, (i+1)*1024).
# Raw Bass (no Tile framework). Per core: 9 pieces per tensor (7 full
# [128 x 2500] tiles + the last tile split 2000+500 cols) cycling through
# 6 SBUF slots. Input pieces stream on the SP HWDGE ring, style pieces on
# the ACT ring; the 16 SDMA engines drain both rings at the ~384 GB/s
# HBM-per-core limit (~53.4us for the 20.5MB shard). Big 1.28MB DMAs are
# load-bearing: half-tile streaming measured ~20% slower. The exact
# [128, 6, 2500] buffer layout is also load-bearing: an 8-slot layout
# measured the scan 20% slower (SBUF bank conflicts between the two
# operand streams at the shifted relative offset).
#
# Compute per piece: the DVE runs the fused subtract+prefix-scan
# (tensor_tensor_scan, the fastest single-pass fp32 primitive at ~2.15
# ns/elem), one drain (the scan does not flush before a dependent strided
# read), and one strided difference for the chunk sums. The
# square+accumulate runs on the otherwise-idle ACT engine (activation
# Square with accum_out into a per-piece partials column), removing two
# DVE ops per piece; cs has one slot per piece so there is no DVE/ACT
# buffer hazard. After the last 500-col piece lands (~62us) only ~1.7us
# of DVE work remains instead of a full 6.3us tile.

import sys

if "/opt/trn_rl_repo" not in sys.path:
    sys.path.insert(0, "/opt/trn_rl_repo")

import numpy as np

import concourse.bass as bass
from concourse import mybir
from concourse.bass_utils import run_bass_kernel_spmd

N_CORES = 8
N_ROWS = 8192          # 16 * 512
K = 2500               # 50 * 50
CHUNK = 100
P = 128
CPL = K // CHUNK                    # 25 chunks per row
ROWS_PER_CORE = N_ROWS // N_CORES   # 1024
N_TILES = ROWS_PER_CORE // P        # 8 tiles of [128 x 2500]
N_BUFS = 6
SPLIT = 2000                        # last tile: [0:2000] + [2000:2500]
PIECES = [(t, 0, K) for t in range(N_TILES - 1)] + [
    (N_TILES - 1, 0, SPLIT),
    (N_TILES - 1, SPLIT, K),
]
N_PIECES = len(PIECES)              # 9
# SBUF slot per piece (by tile) and the compute whose completion frees it.
_SLOT = [t % N_BUFS for (t, _, _) in PIECES]
SCALE = 1.0 / (CHUNK * np.sqrt(CPL))
SCALE2 = float(SCALE * SCALE)

_CACHED_NC = None


def _prev_user(i):
    """Piece index whose compute must finish before piece i's DMA may
    overwrite slot _SLOT[i] (None if the slot is untouched so far)."""
    t = PIECES[i][0]
    prev_t = t - N_BUFS
    if prev_t < 0:
        return None
    for j, (tj, _, _) in enumerate(PIECES):
        if tj == prev_t:
            return j
    return None


def _build_nc():
    nc = bass.Bass(
        "TRN2",
        target_bir_lowering=False,
        debug=False,
        num_devices=N_CORES,
    )
    x = nc.dram_tensor(
        "input", [ROWS_PER_CORE, K], mybir.dt.float32, kind="ExternalInput"
    ).ap()
    s = nc.dram_tensor(
        "style", [ROWS_PER_CORE, K], mybir.dt.float32, kind="ExternalInput"
    ).ap()
    o = nc.dram_tensor(
        "out", [P, N_PIECES], mybir.dt.float32, kind="ExternalOutput"
    ).ap()

    from contextlib import ExitStack

    with ExitStack() as ctx:
        xt = ctx.enter_context(
            nc.sbuf_tensor("xt", [P, N_BUFS, K], mybir.dt.float32)
        )
        st = ctx.enter_context(
            nc.sbuf_tensor("st", [P, N_BUFS, K], mybir.dt.float32)
        )
        # sc col 0 is a permanent zero so chunk sums are one strided sub.
        sc = ctx.enter_context(
            nc.sbuf_tensor("sc", [P, K + 1], mybir.dt.float32)
        )
        # cs one slot per piece: no reuse hazard between DVE and ACT.
        cs = ctx.enter_context(
            nc.sbuf_tensor("cs", [P, N_PIECES, CPL], mybir.dt.float32)
        )
        sq = ctx.enter_context(nc.sbuf_tensor("sq", [P, CPL], mybir.dt.float32))
        partials = ctx.enter_context(
            nc.sbuf_tensor("partials", [P, N_PIECES], mybir.dt.float32)
        )
        # One semaphore per DMA so no completion-ordering assumptions are
        # needed between DMAs on the same ring.
        s_in = [
            ctx.enter_context(nc.semaphore(f"s_in{i}")) for i in range(N_PIECES)
        ]
        s_st = [
            ctx.enter_context(nc.semaphore(f"s_st{i}")) for i in range(N_PIECES)
        ]
        s_d = ctx.enter_context(nc.semaphore("s_d"))
        s_cs = ctx.enter_context(nc.semaphore("s_cs"))
        s_out = ctx.enter_context(nc.semaphore("s_out"))
        block = ctx.enter_context(nc.Block(no_gpsimd_drain=True))

        def src(t_ap, piece):
            t, c0, c1 = piece
            return t_ap[t * P : (t + 1) * P, c0:c1]

        def dst(t_sb, i):
            t, c0, c1 = PIECES[i]
            return t_sb[:, _SLOT[i], c0:c1]

        @block.sync
        def _(sync):
            # Input pieces on the SP HWDGE ring. Slot recycling: piece i may
            # overwrite its slot once the compute of the previous tile in
            # that slot has read its data (s_d counts DVE piece completions).
            for i, piece in enumerate(PIECES):
                p = _prev_user(i)
                if p is not None:
                    sync.wait_ge(s_d, p + 1)
                sync.dma_start(out=dst(xt, i), in_=src(x, piece)).then_inc(
                    s_in[i], 16
                )
            # Ship the per-core partial sums once all pieces are squared.
            sync.wait_ge(s_cs, N_PIECES)
            # No wait on the out-DMA receipt: the 4.6KB write lands in DRAM
            # within ~1us; the engine postamble + NRT teardown give it ample
            # time before the host reads the output.
            sync.dma_start(out=o, in_=partials[:]).then_inc(s_out, 16)

        @block.scalar
        def _(scalar):
            # Style pieces on the ACT HWDGE ring.
            for i, piece in enumerate(PIECES):
                p = _prev_user(i)
                if p is not None:
                    scalar.wait_ge(s_d, p + 1)
                scalar.dma_start(out=dst(st, i), in_=src(s, piece)).then_inc(
                    s_st[i], 16
                )
            # Then the square+accumulate chain: partials[:, i] = sum_c cs^2
            # (SCALE^2 applied on the host).
            for i, piece in enumerate(PIECES):
                nch = (piece[2] - piece[1]) // CHUNK
                scalar.wait_ge(s_d, i + 1)
                nc.scalar.activation(
                    out=sq[:, 0:nch],
                    in_=cs[:, i, 0:nch],
                    func=mybir.ActivationFunctionType.Square,
                    accum_out=partials[:, i : i + 1],
                ).then_inc(s_cs, 1)

        @block.vector
        def _(vector):
            nc.vector.memset(sc[:, 0:1], 0.0)
            for i, piece in enumerate(PIECES):
                w = piece[2] - piece[1]
                nch = w // CHUNK
                vector.wait_ge(s_in[i], 16)
                vector.wait_ge(s_st[i], 16)
                # sc[:, j] = sum_{i<=j} (x - s) over this piece (fp32 state)
                nc.vector.tensor_tensor_scan(
                    out=sc[:, 1 : w + 1],
                    data0=dst(xt, i),
                    data1=dst(st, i),
                    initial=0.0,
                    op0=mybir.AluOpType.add,
                    op1=mybir.AluOpType.subtract,
                )
                # The scan does not flush before a dependent strided read.
                vector.drain()
                # chunk sums: cs[c] = S[100(c+1)] - S[100c]  (S[0] == 0)
                nc.vector.tensor_sub(
                    cs[:, i, 0:nch],
                    sc[:, CHUNK : w + 1 : CHUNK],
                    sc[:, 0:w:CHUNK],
                ).then_inc(s_d, 1)

    return nc


def _get_nc():
    global _CACHED_NC
    if _CACHED_NC is None:
        _CACHED_NC = _build_nc()
    return _CACHED_NC


def run_sharded(input, style, **run_kwargs):
    """Shard, run on 8 cores, return (scalar loss, BassKernelResults)."""
    nc = _get_nc()
    xi = np.ascontiguousarray(np.asarray(input, dtype=np.float32)).reshape(
        N_ROWS, K
    )
    xs = np.ascontiguousarray(np.asarray(style, dtype=np.float32)).reshape(
        N_ROWS, K
    )
    in_maps = [
        {
            "input": xi[i * ROWS_PER_CORE : (i + 1) * ROWS_PER_CORE],
            "style": xs[i * ROWS_PER_CORE : (i + 1) * ROWS_PER_CORE],
        }
        for i in range(N_CORES)
    ]
    res = run_bass_kernel_spmd(nc, in_maps, list(range(N_CORES)), **run_kwargs)
    total = np.float64(0.0)
    for r in res.results:
        total += r["out"].astype(np.float64).sum()
    return np.array(total * SCALE2, dtype=np.float32), res


def kernel(input, style):
    loss, _ = run_sharded(input, style)
    return loss


# revision 16
# speedup vs baseline: 1.1460x; 1.1379x over previous
# Trainium2 Bass kernel for nn_CustomStyleLoss (segment-mean + MSE reduction).
#
# loss = sum_rows mean_chunks( (mean_chunk(input) - mean_chunk(style))^2 )
# with rows = 16*512 = 8192, each row = 50*50 = 2500 elems = 25 chunks of 100.
#
# Data-parallel over the row axis: core i gets rows [i*1024, (i+1)*1024).
# Raw Bass (no Tile framework). Per core: 8 tiles of [128 x 2500] f32 per
# tensor (one row per partition, 10KB DMA lines). Input tiles stream on the
# SP HWDGE ring, style tiles on the ACT ring — two rings together saturate
# the ~384 GB/s HBM-per-core share (~53.4us for the 20.5MB shard). Big
# 1.28MB tile DMAs and the [128, 6, 2500] slot layout are load-bearing:
# half-tile streaming measured ~20% slower (more, shorter descriptors),
# and an 8-slot layout made the DVE scan 20% slower (operand-stream bank
# conflicts at the shifted relative offset).
#
# Compute per tile: the DVE runs the fused subtract+prefix-scan
# (tensor_tensor_scan, the fastest single-pass fp32 primitive at ~2.15
# ns/elem), one drain (the scan does not flush before a dependent strided
# read), and one strided difference for the chunk sums. The
# square+accumulate runs on the otherwise-idle ACT engine (activation
# Square with accum_out into a per-tile partials column), which trims two
# DVE ops (~0.4us) per tile off the serial DVE chain; cs has one slot per
# tile so there is no DVE/ACT buffer hazard. Loss scale is applied on the
# host.

import sys

if "/opt/trn_rl_repo" not in sys.path:
    sys.path.insert(0, "/opt/trn_rl_repo")

import numpy as np

import concourse.bass as bass
from concourse import mybir
from concourse.bass_utils import run_bass_kernel_spmd

N_CORES = 8
N_ROWS = 8192          # 16 * 512
K = 2500               # 50 * 50
CHUNK = 100
P = 128
CPL = K // CHUNK                    # 25 chunks per row
ROWS_PER_CORE = N_ROWS // N_CORES   # 1024
N_TILES = ROWS_PER_CORE // P        # 8
N_BUFS = 6
SCALE = 1.0 / (CHUNK * np.sqrt(CPL))
SCALE2 = float(SCALE * SCALE)

_CACHED_NC = None


def _build_nc():
    nc = bass.Bass(
        "TRN2",
        target_bir_lowering=False,
        debug=False,
        num_devices=N_CORES,
    )
    x = nc.dram_tensor(
        "input", [ROWS_PER_CORE, K], mybir.dt.float32, kind="ExternalInput"
    ).ap()
    s = nc.dram_tensor(
        "style", [ROWS_PER_CORE, K], mybir.dt.float32, kind="ExternalInput"
    ).ap()
    o = nc.dram_tensor(
        "out", [P, N_TILES], mybir.dt.float32, kind="ExternalOutput"
    ).ap()

    def src(t_ap, t):
        return t_ap[t * P : (t + 1) * P, :]

    from contextlib import ExitStack

    with ExitStack() as ctx:
        xt = ctx.enter_context(
            nc.sbuf_tensor("xt", [P, N_BUFS, K], mybir.dt.float32)
        )
        st = ctx.enter_context(
            nc.sbuf_tensor("st", [P, N_BUFS, K], mybir.dt.float32)
        )
        # sc col 0 is a permanent zero so chunk sums are one strided sub.
        sc = ctx.enter_context(
            nc.sbuf_tensor("sc", [P, K + 1], mybir.dt.float32)
        )
        # cs one slot per tile: no reuse hazard between DVE and ACT.
        cs = ctx.enter_context(
            nc.sbuf_tensor("cs", [P, N_TILES, CPL], mybir.dt.float32)
        )
        sq = ctx.enter_context(nc.sbuf_tensor("sq", [P, CPL], mybir.dt.float32))
        partials = ctx.enter_context(
            nc.sbuf_tensor("partials", [P, N_TILES], mybir.dt.float32)
        )
        # One semaphore per DMA so no completion-ordering assumptions are
        # needed between DMAs on the same ring.
        s_in = [
            ctx.enter_context(nc.semaphore(f"s_in{t}")) for t in range(N_TILES)
        ]
        s_st = [
            ctx.enter_context(nc.semaphore(f"s_st{t}")) for t in range(N_TILES)
        ]
        s_d = ctx.enter_context(nc.semaphore("s_d"))
        s_cs = ctx.enter_context(nc.semaphore("s_cs"))
        s_out = ctx.enter_context(nc.semaphore("s_out"))
        block = ctx.enter_context(nc.Block(no_gpsimd_drain=True))

        @block.sync
        def _(sync):
            # Input tiles on the SP HWDGE ring. The first N_BUFS issue
            # immediately; tile t >= N_BUFS reuses slot t % N_BUFS, free once
            # tile t - N_BUFS finished its DVE reads (s_d).
            for t in range(N_TILES):
                if t >= N_BUFS:
                    sync.wait_ge(s_d, t - N_BUFS + 1)
                sync.dma_start(out=xt[:, t % N_BUFS, :], in_=src(x, t)).then_inc(
                    s_in[t], 16
                )
            # Ship the per-core partial sums once all tiles are squared.
            sync.wait_ge(s_cs, N_TILES)
            # No wait on the out-DMA receipt: the 4KB write lands in DRAM
            # within ~1us; the engine postamble + NRT teardown give it ample
            # time before the host reads the output.
            sync.dma_start(out=o, in_=partials[:]).then_inc(s_out, 16)

        @block.scalar
        def _(scalar):
            # Style tiles on the ACT HWDGE ring.
            for t in range(N_TILES):
                if t >= N_BUFS:
                    scalar.wait_ge(s_d, t - N_BUFS + 1)
                scalar.dma_start(out=st[:, t % N_BUFS, :], in_=src(s, t)).then_inc(
                    s_st[t], 16
                )
            # Then the square+accumulate chain: partials[:, t] = sum_c cs^2
            # (SCALE^2 applied on the host).
            for t in range(N_TILES):
                scalar.wait_ge(s_d, t + 1)
                nc.scalar.activation(
                    out=sq[:],
                    in_=cs[:, t, :],
                    func=mybir.ActivationFunctionType.Square,
                    accum_out=partials[:, t : t + 1],
                ).then_inc(s_cs, 1)

        @block.vector
        def _(vector):
            nc.vector.memset(sc[:, 0:1], 0.0)
            for t in range(N_TILES):
                vector.wait_ge(s_in[t], 16)
                vector.wait_ge(s_st[t], 16)
                # sc[:, j] = sum_{i<=j} (xt[:, i] - st[:, i])  (fp32 state)
                nc.vector.tensor_tensor_scan(
                    out=sc[:, 1 : K + 1],
                    data0=xt[:, t % N_BUFS, :],
                    data1=st[:, t % N_BUFS, :],
                    initial=0.0,
                    op0=mybir.AluOpType.add,
                    op1=mybir.AluOpType.subtract,
                )
                # The scan does not flush before a dependent strided read.
                vector.drain()
                # chunk sums: cs[c] = S[100(c+1)] - S[100c]  (S[0] == 0)
                nc.vector.tensor_sub(
                    cs[:, t, :],
                    sc[:, CHUNK : K + 1 : CHUNK],
                    sc[:, 0:K:CHUNK],
                ).then_inc(s_d, 1)

    return nc


def _get_nc():
    global _CACHED_NC
    if _CACHED_NC is None:
        _CACHED_NC = _build_nc()
    return _CACHED_NC


def run_sharded(input, style, **run_kwargs):
    """Shard, run on 8 cores, return (scalar loss, BassKernelResults)."""
    nc = _get_nc()
    xi = np.ascontiguousarray(np.asarray(input, dtype=np.float32)).reshape(
        N_ROWS, K
    )
    xs = np.ascontiguousarray(np.asarray(style, dtype=np.float32)).reshape(
        N_ROWS, K
    )
    in_maps = [
        {
            "input": xi[i * ROWS_PER_CORE : (i + 1) * ROWS_PER_CORE],
            "style": xs[i * ROWS_PER_CORE : (i + 1) * ROWS_PER_CORE],
        }
        for i in range(N_CORES)
    ]
    res = run_bass_kernel_spmd(nc, in_maps, list(range(N_CORES)), **run_kwargs)
    total = np.float64(0.0)
    for r in res.results:
        total += r["out"].astype(np.float64).sum()
    return np.array(total * SCALE2, dtype=np.float32), res


def kernel(input, style):
    loss, _ = run_sharded(input, style)
    return loss
